# revision 4
# baseline (speedup 1.0000x reference)
"""Expert-parallel grouped-GEMM FFN (MoE expert module) for TRN2, 8 NeuronCores.

Problem: xs [16384, 1024] grouped contiguously into 16 experts x 1024 tokens.
Per expert e: y = relu(x @ w1[e].T + b1[e]) @ w2[e].T + b2[e].

Sharding: expert-parallel, 2 experts per core. Each core computes its two
experts' FFN independently; outputs are disjoint row-blocks of the result, so
no collectives are needed.

Per-core kernel (per expert), all matmul operands in bf16 (fp32 PSUM):
  - bf16 runs at the same PE rate as fp32r (1 cycle/row) but its LDWEIGHTS
    uses Fast Weight Load and overlaps cleanly, halves all DMA traffic and
    SBUF footprint (l2 error ~3e-3, well under the 2e-2 gate)
  - keep x^T (D x 1024) resident in SBUF; h^T (all 32 H-chunks) stays
    resident in bf16, so matmul2 runs single 32-long PSUM chains (no
    half-splitting or cross-half DVE pass)
  - matmul1 produces h^T = relu(w1^T.T @ x^T + b1) tile-by-tile over H
  - matmul2 accumulates y^T = w2^T.T @ h^T per output d-chunk, DVE adds b2
    and writes bf16 y straight out via DMA
  - a short chain of garbage warmup matmuls runs during the DMA preamble so
    the PE HAM clock-gate is already at 8/8 when the first real matmul issues
  - weights/activations are laid out host-side (pre-transposed/tiled) so
    every DMA is a contiguous per-partition block
"""

import numpy as np
import ml_dtypes

import concourse.bacc as bacc
import concourse.mybir as mybir
import concourse.tile as tile
from concourse.bass_utils import run_bass_kernel_spmd

P = 128                 # SBUF partitions / PE array dim
D = 1024                # model dim
H = 4096                # hidden dim
E = 16                  # experts
N_TOK = 16384           # total tokens
N_CORES = 8
E_LOC = E // N_CORES    # experts per core = 2
NE = N_TOK // E         # tokens per expert = 1024
DC = D // P             # 8  (d chunks: matmul1 contraction / matmul2 output)
HC = H // P             # 32 (h chunks)
NT = 512                # matmul moving free dim (one PSUM bank of fp32)
NN = NE // NT           # 2  (token tiles per expert)

F32 = mybir.dt.float32
BF16 = mybir.dt.bfloat16
BF16_NP = ml_dtypes.bfloat16

_CACHE = {}


def _build_nc():
    nc = bacc.Bacc(None, target_bir_lowering=False)

    # Host-tiled layouts (see kernel() for the exact index maps):
    #   xt  [s, p, c, n]     = x_e[n, c*128+p]
    #   w1t [s, hi, p, c, j] = w1[e, hi*128+j, c*128+p]
    #   w2t [s, dd, p, k, j] = w2[e, dd*128+j, k*128+p]
    #   b1r [s, p, j]        = b1[e, j*128+p]
    #   b2r [s, p, j]        = b2[e, j*128+p]
    #   yt  [s, p, dd, n]    = y_e[n, dd*128+p]
    xt = nc.dram_tensor("xt", [E_LOC, P, DC, NE], BF16, kind="ExternalInput")
    w1t = nc.dram_tensor("w1t", [E_LOC, HC, P, DC, P], BF16, kind="ExternalInput")
    w2t = nc.dram_tensor("w2t", [E_LOC, DC, P, HC, P], BF16, kind="ExternalInput")
    b1r = nc.dram_tensor("b1r", [E_LOC, P, HC], F32, kind="ExternalInput")
    b2r = nc.dram_tensor("b2r", [E_LOC, P, DC], F32, kind="ExternalInput")
    yt = nc.dram_tensor("yt", [E_LOC, P, DC, NE], BF16, kind="ExternalOutput")

    with tile.TileContext(nc) as tc:
        with (
            tc.tile_pool(name="xpool", bufs=2) as xpool,
            tc.tile_pool(name="hpool", bufs=HC) as hpool,
            tc.tile_pool(name="ypool", bufs=4) as ypool,
            tc.tile_pool(name="w1pool", bufs=9) as w1pool,
            tc.tile_pool(name="w2pool", bufs=3) as w2pool,
            tc.tile_pool(name="cpool", bufs=2) as cpool,
            tc.tile_pool(name="wupool", bufs=1) as wupool,
            tc.tile_pool(name="ps1", bufs=4, space="PSUM") as ps1,
            tc.tile_pool(name="ps2", bufs=4, space="PSUM") as ps2,
        ):
            # PE warmup: ~3.5us of garbage matmuls run while the first
            # weight/x DMAs stream in, so the HAM clock-gate is already at
            # 8/8 (2.4 GHz) when the first real matmul issues.
            wu = wupool.tile([P, P + NT], BF16)
            nc.vector.memset(wu[:], 0.0)
            wacc = ps1.tile([P, NT], F32, name="acc", tag="acc")
            for i in range(9):
                nc.tensor.matmul(
                    wacc[:],
                    wu[:, :P],
                    wu[:, P:],
                    start=(i == 0),
                    stop=(i == 8),
                )

            for s in range(E_LOC):
                # First h-tile's weights lead the sync queue so the PE can
                # start as soon as x chunks stream in; x is split per
                # (chunk, n-tile) with the first n-tile's chunks first — the
                # leading matmul chain needs only those. x issues alternate
                # between the gpsimd and scalar queues so neither the issue
                # overhead nor the weight DMAs on sync serialize them.
                w1_first = w1pool.tile([P, DC, P], BF16, name="w1_t", tag="w1_t")
                nc.sync.dma_start(out=w1_first[:], in_=w1t[s, 0])

                # Biases are tiny; keep them early so the first PSUM
                # eviction never waits.
                b1_t = cpool.tile([P, HC], F32)
                nc.scalar.dma_start(out=b1_t[:], in_=b1r[s])
                b2_t = cpool.tile([P, DC], F32)
                nc.scalar.dma_start(out=b2_t[:], in_=b2r[s])

                x_t = xpool.tile([P, DC, NE], BF16)
                for nt in range(NN):
                    for c in range(DC):
                        eng = nc.gpsimd if c % 2 == 0 else nc.scalar
                        eng.dma_start(
                            out=x_t[:, c, nt * NT : (nt + 1) * NT],
                            in_=xt[s, :, c, nt * NT : (nt + 1) * NT],
                        )

                h_tiles = []
                w1_tiles = []

                def mm1_chain(k, nt):
                    acc = ps1.tile([P, NT], F32, name="acc", tag="acc")
                    for c in range(DC):
                        nc.tensor.matmul(
                            acc[:],
                            w1_tiles[k][:, c, :],
                            x_t[:, c, nt * NT : (nt + 1) * NT],
                            start=(c == 0),
                            stop=(c == DC - 1),
                        )
                    nc.scalar.activation(
                        h_tiles[k][:, nt * NT : (nt + 1) * NT],
                        acc[:],
                        mybir.ActivationFunctionType.Relu,
                        bias=b1_t[:, k : k + 1],
                    )

                # Defer each h-tile's second-n-tile chain by DEFER tiles:
                # at expert start the PE then has 1+DEFER chains of work
                # that need only the first n-tile's x chunks, hiding the
                # second half of the x stream behind compute.
                DEFER = 6
                for k in range(HC):
                    if k == 0:
                        w1_t = w1_first
                    else:
                        w1_t = w1pool.tile([P, DC, P], BF16, name="w1_t", tag="w1_t")
                        nc.sync.dma_start(out=w1_t[:], in_=w1t[s, k])
                    w1_tiles.append(w1_t)
                    h_tiles.append(hpool.tile([P, NE], BF16, name="h_t", tag="h_t"))
                    mm1_chain(k, 0)
                    if k >= DEFER:
                        mm1_chain(k - DEFER, 1)
                for k in range(HC - DEFER, HC):
                    mm1_chain(k, 1)

                for dd in range(DC):
                    w2_t = w2pool.tile([P, HC, P], BF16)
                    nc.sync.dma_start(out=w2_t[:], in_=w2t[s, dd])
                    # Interleave the two n-tile accumulation chains so
                    # consecutive matmuls target alternating PSUM banks
                    # (and reuse the freshly loaded weight chunk).
                    acc2s = [
                        ps2.tile([P, NT], F32, name="acc2", tag="acc2")
                        for _ in range(NN)
                    ]
                    for k in range(HC):
                        for nt in range(NN):
                            nc.tensor.matmul(
                                acc2s[nt][:],
                                w2_t[:, k, :],
                                h_tiles[k][:, nt * NT : (nt + 1) * NT],
                                start=(k == 0),
                                stop=(k == HC - 1),
                            )
                    for nt in range(NN):
                        y_sb = ypool.tile([P, NT], BF16)
                        nc.vector.tensor_scalar_add(
                            y_sb[:], acc2s[nt][:], b2_t[:, dd : dd + 1]
                        )
                        nc.gpsimd.dma_start(
                            out=yt[s, :, dd, nt * NT : (nt + 1) * NT],
                            in_=y_sb[:],
                        )

    nc.finalize()
    return nc


def _get_nc():
    if "nc" not in _CACHE:
        _CACHE["nc"] = _build_nc()
    return _CACHE["nc"]


def _prep_in_maps(xs, w1, b1, w2, b2):
    xs = np.ascontiguousarray(np.asarray(xs, dtype=np.float32))
    w1 = np.asarray(w1, dtype=np.float32)
    b1 = np.asarray(b1, dtype=np.float32)
    w2 = np.asarray(w2, dtype=np.float32)
    b2 = np.asarray(b2, dtype=np.float32)

    x3 = xs.reshape(E, NE, D)
    in_maps = []
    for core in range(N_CORES):
        es = [E_LOC * core + s for s in range(E_LOC)]
        # xt[s, p, c, n] = x_e[n, c*128+p]
        xt = np.stack(
            [x3[e].T.reshape(DC, P, NE).transpose(1, 0, 2) for e in es]
        )
        # w1t[s, hi, p, c, j] = w1[e, hi*128+j, c*128+p]
        w1t = np.stack(
            [w1[e].reshape(HC, P, DC, P).transpose(0, 3, 2, 1) for e in es]
        )
        # w2t[s, dd, p, k, j] = w2[e, dd*128+j, k*128+p]
        w2t = np.stack(
            [w2[e].reshape(DC, P, HC, P).transpose(0, 3, 2, 1) for e in es]
        )
        # b1r[s, p, j] = b1[e, j*128+p]
        b1r = np.stack([b1[e].reshape(HC, P).T for e in es])
        b2r = np.stack([b2[e].reshape(DC, P).T for e in es])
        in_maps.append(
            {
                "xt": np.ascontiguousarray(xt).astype(BF16_NP),
                "w1t": np.ascontiguousarray(w1t).astype(BF16_NP),
                "w2t": np.ascontiguousarray(w2t).astype(BF16_NP),
                "b1r": np.ascontiguousarray(b1r),
                "b2r": np.ascontiguousarray(b2r),
            }
        )
    return in_maps


def _gather(results):
    y = np.empty((N_TOK, D), dtype=np.float32)
    for core in range(N_CORES):
        out = results[core]["yt"]  # [E_LOC, P, DC, NE] bf16
        for s in range(E_LOC):
            e = E_LOC * core + s
            # yt[s, p, dd, n] = y_e[n, dd*128+p]
            y[e * NE : (e + 1) * NE] = (
                out[s].transpose(2, 1, 0).reshape(NE, D).astype(np.float32)
            )
    return y


def _run(in_maps, **kwargs):
    nc = _get_nc()
    return run_bass_kernel_spmd(nc, in_maps, core_ids=list(range(N_CORES)), **kwargs)


def kernel(xs, fwd_expert_count, w1, b1, w2, b2):
    # fwd_expert_count is uniform (N_TOK // E per expert) by construction,
    # matching the reference, which also hardcodes the uniform grouping.
    in_maps = _prep_in_maps(xs, w1, b1, w2, b2)
    res = _run(in_maps)
    return _gather(res.results)


# revision 6
# speedup vs baseline: 1.1886x; 1.1886x over previous
"""Expert-parallel grouped-GEMM FFN (MoE expert module) for TRN2, 8 NeuronCores.

Problem: xs [16384, 1024] grouped contiguously into 16 experts x 1024 tokens.
Per expert e: y = relu(x @ w1[e].T + b1[e]) @ w2[e].T + b2[e].

Sharding: expert-parallel, 2 experts per core. Each core computes its two
experts' FFN independently; outputs are disjoint row-blocks of the result, so
no collectives are needed.

Per-core kernel (per expert), all matmul operands in bf16 (fp32 PSUM):
  - bf16 runs at the same PE rate as fp32r (1 cycle/row) but its LDWEIGHTS
    uses Fast Weight Load and overlaps cleanly, halves all DMA traffic and
    SBUF footprint (l2 error ~3e-3, well under the 2e-2 gate)
  - keep x^T (D x 1024) resident in SBUF; h^T (all 32 H-chunks) stays
    resident in bf16, so matmul2 runs single 32-long PSUM chains (no
    half-splitting or cross-half DVE pass)
  - matmul1 produces h^T = relu(w1^T.T @ x^T + b1) tile-by-tile over H
  - matmul2 accumulates y^T = w2^T.T @ h^T per output d-chunk, DVE adds b2
    and writes bf16 y straight out via DMA
  - a short chain of garbage warmup matmuls runs during the DMA preamble so
    the PE HAM clock-gate is already at 8/8 when the first real matmul issues
  - weights/activations are laid out host-side (pre-transposed/tiled) so
    every DMA is a contiguous per-partition block
"""

import numpy as np
import ml_dtypes

import concourse.bacc as bacc
import concourse.mybir as mybir
import concourse.tile as tile
from concourse.bass_utils import run_bass_kernel_spmd

P = 128                 # SBUF partitions / PE array dim
D = 1024                # model dim
H = 4096                # hidden dim
E = 16                  # experts
N_TOK = 16384           # total tokens
N_CORES = 8
E_LOC = E // N_CORES    # experts per core = 2
NE = N_TOK // E         # tokens per expert = 1024
DC = D // P             # 8  (d chunks: matmul1 contraction / matmul2 output)
HC = H // P             # 32 (h chunks)
NT = 512                # matmul moving free dim (one PSUM bank of fp32)
NN = NE // NT           # 2  (token tiles per expert)

F32 = mybir.dt.float32
F32R = mybir.dt.float32r
F16 = mybir.dt.float16
BF16_NP = ml_dtypes.bfloat16

_CACHE = {}


def _build_nc():
    nc = bacc.Bacc(None, target_bir_lowering=False)

    # Host-tiled layouts (see kernel() for the exact index maps):
    #   xt  [s, p, c, n]     = x_e[n, c*128+p]
    #   w1t [s, hi, p, c, j] = w1[e, hi*128+j, c*128+p]
    #   w2t [s, dd, p, k, j] = w2[e, dd*128+j, k*128+p]
    #   b1r [s, p, j]        = b1[e, j*128+p]
    #   b2r [s, p, j]        = b2[e, j*128+p]
    #   yt  [s, p, dd, n]    = y_e[n, dd*128+p]
    xt = nc.dram_tensor("xt", [E_LOC, P, DC, NE], F16, kind="ExternalInput")
    w1t = nc.dram_tensor("w1t", [E_LOC, HC, P, DC, P], F16, kind="ExternalInput")
    w2t = nc.dram_tensor("w2t", [E_LOC, DC, P, HC, P], F16, kind="ExternalInput")
    b1r = nc.dram_tensor("b1r", [E_LOC, P, HC], F32, kind="ExternalInput")
    b2r = nc.dram_tensor("b2r", [E_LOC, P, DC], F32, kind="ExternalInput")
    yt = nc.dram_tensor("yt", [E_LOC, P, DC, NE], F16, kind="ExternalOutput")

    with tile.TileContext(nc) as tc:
        with (
            tc.tile_pool(name="xpool", bufs=2) as xpool,
            tc.tile_pool(name="hpool", bufs=HC) as hpool,
            tc.tile_pool(name="ypool", bufs=4) as ypool,
            tc.tile_pool(name="w1pool", bufs=9) as w1pool,
            tc.tile_pool(name="w2pool", bufs=2) as w2pool,
            tc.tile_pool(name="cpool", bufs=2) as cpool,
            tc.tile_pool(name="wupool", bufs=1) as wupool,
            tc.tile_pool(name="ps1", bufs=4, space="PSUM") as ps1,
            tc.tile_pool(name="ps2", bufs=4, space="PSUM") as ps2,
        ):
            # PE warmup: ~3.5us of garbage matmuls run while the first
            # weight/x DMAs stream in, so the HAM clock-gate is already at
            # 8/8 (2.4 GHz) when the first real matmul issues.
            wu = wupool.tile([P, P + NT], F16)
            nc.vector.memset(wu[:], 0.0)
            wacc = ps1.tile([P, NT], F32, name="acc", tag="acc")
            for i in range(9):
                nc.tensor.matmul(
                    wacc[:],
                    wu[:, :P],
                    wu[:, P:],
                    start=(i == 0),
                    stop=(i == 8),
                )

            for s in range(E_LOC):
                # First h-tile's weights lead the sync queue so the PE can
                # start as soon as x chunks stream in; x is split per
                # (chunk, n-tile) with the first n-tile's chunks first — the
                # leading matmul chain needs only those. x issues alternate
                # between the gpsimd and scalar queues so neither the issue
                # overhead nor the weight DMAs on sync serialize them.
                w1_first = w1pool.tile([P, DC, P], F16, name="w1_t", tag="w1_t")
                nc.sync.dma_start(out=w1_first[:], in_=w1t[s, 0])

                # Biases are tiny; keep them early so the first PSUM
                # eviction never waits.
                b1_t = cpool.tile([P, HC], F32)
                nc.scalar.dma_start(out=b1_t[:], in_=b1r[s])
                b2_t = cpool.tile([P, DC], F32)
                nc.scalar.dma_start(out=b2_t[:], in_=b2r[s])

                x_t = xpool.tile([P, DC, NE], F16)
                for nt in range(NN):
                    for c in range(DC):
                        eng = nc.gpsimd if c % 2 == 0 else nc.scalar
                        eng.dma_start(
                            out=x_t[:, c, nt * NT : (nt + 1) * NT],
                            in_=xt[s, :, c, nt * NT : (nt + 1) * NT],
                        )

                h_tiles = []
                w1_tiles = []

                def mm1_chain(k, nt):
                    acc = ps1.tile([P, NT], F32, name="acc", tag="acc")
                    for c in range(DC):
                        nc.tensor.matmul(
                            acc[:],
                            w1_tiles[k][:, c, :],
                            x_t[:, c, nt * NT : (nt + 1) * NT],
                            start=(c == 0),
                            stop=(c == DC - 1),
                        )
                    nc.scalar.activation(
                        h_tiles[k][:, nt * NT : (nt + 1) * NT],
                        acc[:],
                        mybir.ActivationFunctionType.Relu,
                        bias=b1_t[:, k : k + 1],
                    )

                # Defer each h-tile's second-n-tile chain by DEFER tiles:
                # at expert start the PE then has 1+DEFER chains of work
                # that need only the first n-tile's x chunks, hiding the
                # second half of the x stream behind compute.
                DEFER = 6
                for k in range(HC):
                    if k == 0:
                        w1_t = w1_first
                    else:
                        w1_t = w1pool.tile([P, DC, P], F16, name="w1_t", tag="w1_t")
                        nc.sync.dma_start(out=w1_t[:], in_=w1t[s, k])
                    w1_tiles.append(w1_t)
                    h_tiles.append(hpool.tile([P, NE], F16, name="h_t", tag="h_t"))
                    mm1_chain(k, 0)
                    if k >= DEFER:
                        mm1_chain(k - DEFER, 1)
                for k in range(HC - DEFER, HC):
                    mm1_chain(k, 1)

                for dd in range(DC):
                    w2_t = w2pool.tile([P, HC, P], F16)
                    nc.sync.dma_start(out=w2_t[:], in_=w2t[s, dd])
                    # Interleave the two n-tile accumulation chains so
                    # consecutive matmuls target alternating PSUM banks
                    # (and reuse the freshly loaded weight chunk).
                    acc2s = [
                        ps2.tile([P, NT], F32, name="acc2", tag="acc2")
                        for _ in range(NN)
                    ]
                    for k in range(HC):
                        for nt in range(NN):
                            nc.tensor.matmul(
                                acc2s[nt][:],
                                w2_t[:, k, :],
                                h_tiles[k][:, nt * NT : (nt + 1) * NT],
                                start=(k == 0),
                                stop=(k == HC - 1),
                            )
                    for nt in range(NN):
                        y_sb = ypool.tile([P, NT], F16)
                        nc.vector.tensor_scalar_add(
                            y_sb[:], acc2s[nt][:], b2_t[:, dd : dd + 1]
                        )
                        nc.gpsimd.dma_start(
                            out=yt[s, :, dd, nt * NT : (nt + 1) * NT],
                            in_=y_sb[:],
                        )

    nc.finalize()
    return nc


def _get_nc():
    if "nc" not in _CACHE:
        _CACHE["nc"] = _build_nc()
    return _CACHE["nc"]


def _prep_in_maps(xs, w1, b1, w2, b2):
    xs = np.ascontiguousarray(np.asarray(xs, dtype=np.float32))
    w1 = np.asarray(w1, dtype=np.float32)
    b1 = np.asarray(b1, dtype=np.float32)
    w2 = np.asarray(w2, dtype=np.float32)
    b2 = np.asarray(b2, dtype=np.float32)

    x3 = xs.reshape(E, NE, D)
    in_maps = []
    for core in range(N_CORES):
        es = [E_LOC * core + s for s in range(E_LOC)]
        # xt[s, p, c, n] = x_e[n, c*128+p]
        xt = np.stack(
            [x3[e].T.reshape(DC, P, NE).transpose(1, 0, 2) for e in es]
        )
        # w1t[s, hi, p, c, j] = w1[e, hi*128+j, c*128+p]
        w1t = np.stack(
            [w1[e].reshape(HC, P, DC, P).transpose(0, 3, 2, 1) for e in es]
        )
        # w2t[s, dd, p, k, j] = w2[e, dd*128+j, k*128+p]
        w2t = np.stack(
            [w2[e].reshape(DC, P, HC, P).transpose(0, 3, 2, 1) for e in es]
        )
        # b1r[s, p, j] = b1[e, j*128+p]
        b1r = np.stack([b1[e].reshape(HC, P).T for e in es])
        b2r = np.stack([b2[e].reshape(DC, P).T for e in es])
        in_maps.append(
            {
                "xt": np.ascontiguousarray(xt).astype(np.float16),
                "w1t": np.ascontiguousarray(w1t).astype(np.float16),
                "w2t": np.ascontiguousarray(w2t).astype(np.float16),
                "b1r": np.ascontiguousarray(b1r),
                "b2r": np.ascontiguousarray(b2r),
            }
        )
    return in_maps


def _gather(results):
    y = np.empty((N_TOK, D), dtype=np.float32)
    for core in range(N_CORES):
        out = results[core]["yt"]  # [E_LOC, P, DC, NE] bf16
        for s in range(E_LOC):
            e = E_LOC * core + s
            # yt[s, p, dd, n] = y_e[n, dd*128+p]
            y[e * NE : (e + 1) * NE] = (
                out[s].transpose(2, 1, 0).reshape(NE, D).astype(np.float32)
            )
    return y


def _run(in_maps, **kwargs):
    nc = _get_nc()
    return run_bass_kernel_spmd(nc, in_maps, core_ids=list(range(N_CORES)), **kwargs)


def kernel(xs, fwd_expert_count, w1, b1, w2, b2):
    # fwd_expert_count is uniform (N_TOK // E per expert) by construction,
    # matching the reference, which also hardcodes the uniform grouping.
    in_maps = _prep_in_maps(xs, w1, b1, w2, b2)
    res = _run(in_maps)
    return _gather(res.results)


# revision 8
# speedup vs baseline: 1.1912x; 1.0022x over previous
"""Expert-parallel grouped-GEMM FFN (MoE expert module) for TRN2, 8 NeuronCores.

Problem: xs [16384, 1024] grouped contiguously into 16 experts x 1024 tokens.
Per expert e: y = relu(x @ w1[e].T + b1[e]) @ w2[e].T + b2[e].

Sharding: expert-parallel, 2 experts per core. Each core computes its two
experts' FFN independently; outputs are disjoint row-blocks of the result, so
no collectives are needed.

Per-core kernel (per expert), all matmul operands in bf16 (fp32 PSUM):
  - bf16 runs at the same PE rate as fp32r (1 cycle/row) but its LDWEIGHTS
    uses Fast Weight Load and overlaps cleanly, halves all DMA traffic and
    SBUF footprint (l2 error ~3e-3, well under the 2e-2 gate)
  - keep x^T (D x 1024) resident in SBUF; h^T (all 32 H-chunks) stays
    resident in bf16, so matmul2 runs single 32-long PSUM chains (no
    half-splitting or cross-half DVE pass)
  - matmul1 produces h^T = relu(w1^T.T @ x^T + b1) tile-by-tile over H
  - matmul2 accumulates y^T = w2^T.T @ h^T per output d-chunk, DVE adds b2
    and writes bf16 y straight out via DMA
  - a short chain of garbage warmup matmuls runs during the DMA preamble so
    the PE HAM clock-gate is already at 8/8 when the first real matmul issues
  - weights/activations are laid out host-side (pre-transposed/tiled) so
    every DMA is a contiguous per-partition block
"""

import numpy as np
import ml_dtypes

import concourse.bacc as bacc
import concourse.mybir as mybir
import concourse.tile as tile
from concourse.bass_utils import run_bass_kernel_spmd

P = 128                 # SBUF partitions / PE array dim
D = 1024                # model dim
H = 4096                # hidden dim
E = 16                  # experts
N_TOK = 16384           # total tokens
N_CORES = 8
E_LOC = E // N_CORES    # experts per core = 2
NE = N_TOK // E         # tokens per expert = 1024
DC = D // P             # 8  (d chunks: matmul1 contraction / matmul2 output)
HC = H // P             # 32 (h chunks)
NT = 512                # matmul moving free dim (one PSUM bank of fp32)
NN = NE // NT           # 2  (token tiles per expert)

F32 = mybir.dt.float32
F32R = mybir.dt.float32r
F16 = mybir.dt.float16
BF16_NP = ml_dtypes.bfloat16

_CACHE = {}


def _build_nc():
    nc = bacc.Bacc(None, target_bir_lowering=False)

    # Host-tiled layouts (see kernel() for the exact index maps):
    #   xt  [s, p, c, n]     = x_e[n, c*128+p]
    #   w1t [s, hi, p, c, j] = w1[e, hi*128+j, c*128+p]
    #   w2t [s, dd, p, k, j] = w2[e, dd*128+j, k*128+p]
    #   b1r [s, p, j]        = b1[e, j*128+p]
    #   b2r [s, p, j]        = b2[e, j*128+p]
    #   yt  [s, p, dd, n]    = y_e[n, dd*128+p]
    xt = nc.dram_tensor("xt", [E_LOC, P, DC, NE], F16, kind="ExternalInput")
    w1t = nc.dram_tensor("w1t", [E_LOC, HC, P, DC, P], F16, kind="ExternalInput")
    w2t = nc.dram_tensor("w2t", [E_LOC, DC, P, HC, P], F16, kind="ExternalInput")
    b1r = nc.dram_tensor("b1r", [E_LOC, P, HC], F32, kind="ExternalInput")
    b2r = nc.dram_tensor("b2r", [E_LOC, P, DC], F32, kind="ExternalInput")
    yt = nc.dram_tensor("yt", [E_LOC, P, DC, NE], F16, kind="ExternalOutput")

    with tile.TileContext(nc) as tc:
        with (
            tc.tile_pool(name="xpool", bufs=2) as xpool,
            tc.tile_pool(name="hpool", bufs=HC) as hpool,
            tc.tile_pool(name="ypool", bufs=4) as ypool,
            tc.tile_pool(name="w1pool", bufs=9) as w1pool,
            tc.tile_pool(name="w2pool", bufs=2) as w2pool,
            tc.tile_pool(name="cpool", bufs=2) as cpool,
            tc.tile_pool(name="wupool", bufs=1) as wupool,
            tc.tile_pool(name="ps1", bufs=4, space="PSUM") as ps1,
            tc.tile_pool(name="ps2", bufs=4, space="PSUM") as ps2,
        ):
            # PE warmup: ~3.5us of garbage matmuls run while the first
            # weight/x DMAs stream in, so the HAM clock-gate is already at
            # 8/8 (2.4 GHz) when the first real matmul issues.
            wu = wupool.tile([P, P + NT], F16)
            nc.vector.memset(wu[:], 0.0)
            wacc = ps1.tile([P, NT], F32, name="acc", tag="acc")
            N_WU = 12
            for i in range(N_WU):
                nc.tensor.matmul(
                    wacc[:],
                    wu[:, :P],
                    wu[:, P:],
                    start=(i == 0),
                    stop=(i == N_WU - 1),
                )

            for s in range(E_LOC):
                # First h-tile's weights lead the sync queue so the PE can
                # start as soon as x chunks stream in; x is split per
                # (chunk, n-tile) with the first n-tile's chunks first — the
                # leading matmul chain needs only those. x issues alternate
                # between the gpsimd and scalar queues so neither the issue
                # overhead nor the weight DMAs on sync serialize them.
                w1_first = w1pool.tile([P, DC, P], F16, name="w1_t", tag="w1_t")
                nc.sync.dma_start(out=w1_first[:], in_=w1t[s, 0])

                # Biases are tiny; keep them early so the first PSUM
                # eviction never waits.
                b1_t = cpool.tile([P, HC], F32)
                nc.scalar.dma_start(out=b1_t[:], in_=b1r[s])
                b2_t = cpool.tile([P, DC], F32)
                nc.scalar.dma_start(out=b2_t[:], in_=b2r[s])

                # First n-tile's x chunks go 3-wide across the sync, gpsimd
                # and scalar queues — on sync they sit AHEAD of the w1 tile
                # prefetch, so the leading chains' x is not starved by the
                # weight stream at expert start. The second n-tile streams on
                # gpsimd/scalar only (sync continues with w1 tiles).
                x_t = xpool.tile([P, DC, NE], F16)
                for nt in range(NN):
                    for c in range(DC):
                        if nt == 0:
                            eng = (nc.sync, nc.gpsimd, nc.scalar)[c % 3]
                        else:
                            eng = nc.gpsimd if c % 2 == 0 else nc.scalar
                        eng.dma_start(
                            out=x_t[:, c, nt * NT : (nt + 1) * NT],
                            in_=xt[s, :, c, nt * NT : (nt + 1) * NT],
                        )

                h_tiles = []
                w1_tiles = []

                def mm1_chain(k, nt):
                    acc = ps1.tile([P, NT], F32, name="acc", tag="acc")
                    for c in range(DC):
                        nc.tensor.matmul(
                            acc[:],
                            w1_tiles[k][:, c, :],
                            x_t[:, c, nt * NT : (nt + 1) * NT],
                            start=(c == 0),
                            stop=(c == DC - 1),
                        )
                    nc.scalar.activation(
                        h_tiles[k][:, nt * NT : (nt + 1) * NT],
                        acc[:],
                        mybir.ActivationFunctionType.Relu,
                        bias=b1_t[:, k : k + 1],
                    )

                # Defer each h-tile's second-n-tile chain by DEFER tiles:
                # at expert start the PE then has 1+DEFER chains of work
                # that need only the first n-tile's x chunks, hiding the
                # second half of the x stream behind compute.
                DEFER = 6
                for k in range(HC):
                    if k == 0:
                        w1_t = w1_first
                    else:
                        w1_t = w1pool.tile([P, DC, P], F16, name="w1_t", tag="w1_t")
                        nc.sync.dma_start(out=w1_t[:], in_=w1t[s, k])
                    w1_tiles.append(w1_t)
                    h_tiles.append(hpool.tile([P, NE], F16, name="h_t", tag="h_t"))
                    mm1_chain(k, 0)
                    if k >= DEFER:
                        mm1_chain(k - DEFER, 1)
                for k in range(HC - DEFER, HC):
                    mm1_chain(k, 1)

                for dd in range(DC):
                    w2_t = w2pool.tile([P, HC, P], F16)
                    nc.sync.dma_start(out=w2_t[:], in_=w2t[s, dd])
                    # Interleave the two n-tile accumulation chains so
                    # consecutive matmuls target alternating PSUM banks
                    # (and reuse the freshly loaded weight chunk).
                    acc2s = [
                        ps2.tile([P, NT], F32, name="acc2", tag="acc2")
                        for _ in range(NN)
                    ]
                    for k in range(HC):
                        for nt in range(NN):
                            nc.tensor.matmul(
                                acc2s[nt][:],
                                w2_t[:, k, :],
                                h_tiles[k][:, nt * NT : (nt + 1) * NT],
                                start=(k == 0),
                                stop=(k == HC - 1),
                            )
                    for nt in range(NN):
                        y_sb = ypool.tile([P, NT], F16)
                        nc.vector.tensor_scalar_add(
                            y_sb[:], acc2s[nt][:], b2_t[:, dd : dd + 1]
                        )
                        nc.gpsimd.dma_start(
                            out=yt[s, :, dd, nt * NT : (nt + 1) * NT],
                            in_=y_sb[:],
                        )

    nc.finalize()
    return nc


def _get_nc():
    if "nc" not in _CACHE:
        _CACHE["nc"] = _build_nc()
    return _CACHE["nc"]


def _prep_in_maps(xs, w1, b1, w2, b2):
    xs = np.ascontiguousarray(np.asarray(xs, dtype=np.float32))
    w1 = np.asarray(w1, dtype=np.float32)
    b1 = np.asarray(b1, dtype=np.float32)
    w2 = np.asarray(w2, dtype=np.float32)
    b2 = np.asarray(b2, dtype=np.float32)

    x3 = xs.reshape(E, NE, D)
    in_maps = []
    for core in range(N_CORES):
        es = [E_LOC * core + s for s in range(E_LOC)]
        # xt[s, p, c, n] = x_e[n, c*128+p]
        xt = np.stack(
            [x3[e].T.reshape(DC, P, NE).transpose(1, 0, 2) for e in es]
        )
        # w1t[s, hi, p, c, j] = w1[e, hi*128+j, c*128+p]
        w1t = np.stack(
            [w1[e].reshape(HC, P, DC, P).transpose(0, 3, 2, 1) for e in es]
        )
        # w2t[s, dd, p, k, j] = w2[e, dd*128+j, k*128+p]
        w2t = np.stack(
            [w2[e].reshape(DC, P, HC, P).transpose(0, 3, 2, 1) for e in es]
        )
        # b1r[s, p, j] = b1[e, j*128+p]
        b1r = np.stack([b1[e].reshape(HC, P).T for e in es])
        b2r = np.stack([b2[e].reshape(DC, P).T for e in es])
        in_maps.append(
            {
                "xt": np.ascontiguousarray(xt).astype(np.float16),
                "w1t": np.ascontiguousarray(w1t).astype(np.float16),
                "w2t": np.ascontiguousarray(w2t).astype(np.float16),
                "b1r": np.ascontiguousarray(b1r),
                "b2r": np.ascontiguousarray(b2r),
            }
        )
    return in_maps


def _gather(results):
    y = np.empty((N_TOK, D), dtype=np.float32)
    for core in range(N_CORES):
        out = results[core]["yt"]  # [E_LOC, P, DC, NE] bf16
        for s in range(E_LOC):
            e = E_LOC * core + s
            # yt[s, p, dd, n] = y_e[n, dd*128+p]
            y[e * NE : (e + 1) * NE] = (
                out[s].transpose(2, 1, 0).reshape(NE, D).astype(np.float32)
            )
    return y


def _run(in_maps, **kwargs):
    nc = _get_nc()
    return run_bass_kernel_spmd(nc, in_maps, core_ids=list(range(N_CORES)), **kwargs)


def kernel(xs, fwd_expert_count, w1, b1, w2, b2):
    # fwd_expert_count is uniform (N_TOK // E per expert) by construction,
    # matching the reference, which also hardcodes the uniform grouping.
    in_maps = _prep_in_maps(xs, w1, b1, w2, b2)
    res = _run(in_maps)
    return _gather(res.results)


# revision 12
# speedup vs baseline: 1.1979x; 1.0056x over previous
"""Expert-parallel grouped-GEMM FFN (MoE expert module) for TRN2, 8 NeuronCores.

Problem: xs [16384, 1024] grouped contiguously into 16 experts x 1024 tokens.
Per expert e: y = relu(x @ w1[e].T + b1[e]) @ w2[e].T + b2[e].

Sharding: expert-parallel, 2 experts per core. Each core computes its two
experts' FFN independently; outputs are disjoint row-blocks of the result, so
no collectives are needed.

Per-core kernel (per expert), all matmul operands in bf16 (fp32 PSUM):
  - bf16 runs at the same PE rate as fp32r (1 cycle/row) but its LDWEIGHTS
    uses Fast Weight Load and overlaps cleanly, halves all DMA traffic and
    SBUF footprint (l2 error ~3e-3, well under the 2e-2 gate)
  - keep x^T (D x 1024) resident in SBUF; h^T (all 32 H-chunks) stays
    resident in bf16, so matmul2 runs single 32-long PSUM chains (no
    half-splitting or cross-half DVE pass)
  - matmul1 produces h^T = relu(w1^T.T @ x^T + b1) tile-by-tile over H
  - matmul2 accumulates y^T = w2^T.T @ h^T per output d-chunk, DVE adds b2
    and writes bf16 y straight out via DMA
  - a short chain of garbage warmup matmuls runs during the DMA preamble so
    the PE HAM clock-gate is already at 8/8 when the first real matmul issues
  - weights/activations are laid out host-side (pre-transposed/tiled) so
    every DMA is a contiguous per-partition block
"""

import numpy as np
import ml_dtypes

import concourse.bacc as bacc
import concourse.mybir as mybir
import concourse.tile as tile
from concourse.bass_utils import run_bass_kernel_spmd

P = 128                 # SBUF partitions / PE array dim
D = 1024                # model dim
H = 4096                # hidden dim
E = 16                  # experts
N_TOK = 16384           # total tokens
N_CORES = 8
E_LOC = E // N_CORES    # experts per core = 2
NE = N_TOK // E         # tokens per expert = 1024
DC = D // P             # 8  (d chunks: matmul1 contraction / matmul2 output)
HC = H // P             # 32 (h chunks)
NT = 512                # matmul moving free dim (one PSUM bank of fp32)
NN = NE // NT           # 2  (token tiles per expert)

F32 = mybir.dt.float32
F32R = mybir.dt.float32r
F16 = mybir.dt.float16
BF16_NP = ml_dtypes.bfloat16

_CACHE = {}


def _build_nc():
    nc = bacc.Bacc(None, target_bir_lowering=False)

    # Host-tiled layouts (see kernel() for the exact index maps). The DMA
    # engines are packet-rate limited (~40-50 packets/us per queue), so all
    # large streams keep >=4KB contiguous per partition per packet.
    #   xt  [s, nt, p, c, n]    = x_e[nt*512+n, c*128+p]
    #   w1t [s, g, p, u, c, j]  = w1[e, (2g+u)*128+j, c*128+p]
    #   w2t [s, dd, p, k, j]    = w2[e, dd*128+j, k*128+p]
    #   b1r [s, p, j]           = b1[e, j*128+p]
    #   b2r [s, p, j]           = b2[e, j*128+p]
    #   yt  [s, p, dd, n]       = y_e[n, dd*128+p]
    xt = nc.dram_tensor("xt", [E_LOC, NN, P, DC, NT], F16, kind="ExternalInput")
    w1t = nc.dram_tensor(
        "w1t", [E_LOC, HC // 2, P, 2, DC, P], F16, kind="ExternalInput"
    )
    w2t = nc.dram_tensor("w2t", [E_LOC, DC, P, HC, P], F16, kind="ExternalInput")
    b1r = nc.dram_tensor("b1r", [E_LOC, P, HC], F32, kind="ExternalInput")
    b2r = nc.dram_tensor("b2r", [E_LOC, P, DC], F32, kind="ExternalInput")
    yt = nc.dram_tensor("yt", [E_LOC, P, DC, NE], F16, kind="ExternalOutput")

    with tile.TileContext(nc) as tc:
        with (
            tc.tile_pool(name="xpool", bufs=2) as xpool,
            tc.tile_pool(name="hpool", bufs=HC) as hpool,
            tc.tile_pool(name="ypool", bufs=4) as ypool,
            tc.tile_pool(name="w1pool", bufs=6) as w1pool,
            tc.tile_pool(name="w2pool", bufs=3) as w2pool,
            tc.tile_pool(name="cpool", bufs=2) as cpool,
            tc.tile_pool(name="wupool", bufs=1) as wupool,
            tc.tile_pool(name="ps1", bufs=4, space="PSUM") as ps1,
            tc.tile_pool(name="ps2", bufs=4, space="PSUM") as ps2,
        ):
            # PE warmup: ~3.5us of garbage matmuls run while the first
            # weight/x DMAs stream in, so the HAM clock-gate is already at
            # 8/8 (2.4 GHz) when the first real matmul issues.
            wu = wupool.tile([P, P + NT], F16)
            nc.vector.memset(wu[:], 0.0)
            wacc = ps1.tile([P, NT], F32, name="acc", tag="acc")
            N_WU = 12
            for i in range(N_WU):
                nc.tensor.matmul(
                    wacc[:],
                    wu[:, :P],
                    wu[:, P:],
                    start=(i == 0),
                    stop=(i == N_WU - 1),
                )

            for s in range(E_LOC):
                # First h-tile-pair's weights lead the sync queue so the PE
                # can start as soon as x streams in. The first n-tile's x
                # goes in two 4KB-per-partition blocks on sync+gpsimd (on
                # sync it sits AHEAD of the w1 tile prefetch, so the leading
                # chains' x is not starved by the weight stream); the second
                # n-tile follows on scalar+gpsimd.
                w1_first = w1pool.tile([P, 2, DC, P], F16, name="w1_t", tag="w1_t")
                nc.sync.dma_start(out=w1_first[:], in_=w1t[s, 0])

                # Biases are tiny; keep them early so the first PSUM
                # eviction never waits.
                b1_t = cpool.tile([P, HC], F32)
                nc.scalar.dma_start(out=b1_t[:], in_=b1r[s])
                b2_t = cpool.tile([P, DC], F32)
                nc.scalar.dma_start(out=b2_t[:], in_=b2r[s])

                x_t = xpool.tile([P, NN, DC, NT], F16)
                HD = DC // 2
                nc.sync.dma_start(out=x_t[:, 0, :HD], in_=xt[s, 0, :, :HD])
                nc.gpsimd.dma_start(out=x_t[:, 0, HD:], in_=xt[s, 0, :, HD:])
                nc.scalar.dma_start(out=x_t[:, 1, :HD], in_=xt[s, 1, :, :HD])
                nc.gpsimd.dma_start(out=x_t[:, 1, HD:], in_=xt[s, 1, :, HD:])

                h_tiles = []
                w1_aps = []

                def mm1_chain(k, nt):
                    acc = ps1.tile([P, NT], F32, name="acc", tag="acc")
                    for c in range(DC):
                        nc.tensor.matmul(
                            acc[:],
                            w1_aps[k][:, c, :],
                            x_t[:, nt, c, :],
                            start=(c == 0),
                            stop=(c == DC - 1),
                        )
                    nc.scalar.activation(
                        h_tiles[k][:, nt * NT : (nt + 1) * NT],
                        acc[:],
                        mybir.ActivationFunctionType.Relu,
                        bias=b1_t[:, k : k + 1],
                    )

                # Defer each h-tile's second-n-tile chain by DEFER tiles:
                # at expert start the PE then has 1+DEFER chains of work
                # that need only the first n-tile's x, hiding the second
                # half of the x stream behind compute.
                DEFER = 6
                for g in range(HC // 2):
                    if g == 0:
                        w1_t = w1_first
                    else:
                        w1_t = w1pool.tile(
                            [P, 2, DC, P], F16, name="w1_t", tag="w1_t"
                        )
                        nc.sync.dma_start(out=w1_t[:], in_=w1t[s, g])
                    for u in range(2):
                        k = 2 * g + u
                        w1_aps.append(w1_t[:, u])
                        h_tiles.append(
                            hpool.tile([P, NE], F16, name="h_t", tag="h_t")
                        )
                        mm1_chain(k, 0)
                        if k >= DEFER:
                            mm1_chain(k - DEFER, 1)
                for k in range(HC - DEFER, HC):
                    mm1_chain(k, 1)

                for dd in range(DC):
                    w2_t = w2pool.tile([P, HC, P], F16)
                    nc.sync.dma_start(out=w2_t[:], in_=w2t[s, dd])
                    # Interleave the two n-tile accumulation chains so
                    # consecutive matmuls target alternating PSUM banks
                    # (and reuse the freshly loaded weight chunk).
                    acc2s = [
                        ps2.tile([P, NT], F32, name="acc2", tag="acc2")
                        for _ in range(NN)
                    ]
                    for k in range(HC):
                        for nt in range(NN):
                            nc.tensor.matmul(
                                acc2s[nt][:],
                                w2_t[:, k, :],
                                h_tiles[k][:, nt * NT : (nt + 1) * NT],
                                start=(k == 0),
                                stop=(k == HC - 1),
                            )
                    for nt in range(NN):
                        y_sb = ypool.tile([P, NT], F16)
                        nc.vector.tensor_scalar_add(
                            y_sb[:], acc2s[nt][:], b2_t[:, dd : dd + 1]
                        )
                        nc.gpsimd.dma_start(
                            out=yt[s, :, dd, nt * NT : (nt + 1) * NT],
                            in_=y_sb[:],
                        )

    nc.finalize()
    return nc


def _get_nc():
    if "nc" not in _CACHE:
        _CACHE["nc"] = _build_nc()
    return _CACHE["nc"]


def _prep_in_maps(xs, w1, b1, w2, b2):
    xs = np.ascontiguousarray(np.asarray(xs, dtype=np.float32))
    w1 = np.asarray(w1, dtype=np.float32)
    b1 = np.asarray(b1, dtype=np.float32)
    w2 = np.asarray(w2, dtype=np.float32)
    b2 = np.asarray(b2, dtype=np.float32)

    x3 = xs.reshape(E, NE, D)
    in_maps = []
    for core in range(N_CORES):
        es = [E_LOC * core + s for s in range(E_LOC)]
        # xt[s, nt, p, c, n] = x_e[nt*512+n, c*128+p]
        xt = np.stack(
            [x3[e].reshape(NN, NT, DC, P).transpose(0, 3, 2, 1) for e in es]
        )
        # w1t[s, g, p, u, c, j] = w1[e, (2g+u)*128+j, c*128+p]
        w1t = np.stack(
            [
                w1[e].reshape(HC // 2, 2, P, DC, P).transpose(0, 4, 1, 3, 2)
                for e in es
            ]
        )
        # w2t[s, dd, p, k, j] = w2[e, dd*128+j, k*128+p]
        w2t = np.stack(
            [w2[e].reshape(DC, P, HC, P).transpose(0, 3, 2, 1) for e in es]
        )
        # b1r[s, p, j] = b1[e, j*128+p]
        b1r = np.stack([b1[e].reshape(HC, P).T for e in es])
        b2r = np.stack([b2[e].reshape(DC, P).T for e in es])
        in_maps.append(
            {
                "xt": np.ascontiguousarray(xt).astype(np.float16),
                "w1t": np.ascontiguousarray(w1t).astype(np.float16),
                "w2t": np.ascontiguousarray(w2t).astype(np.float16),
                "b1r": np.ascontiguousarray(b1r),
                "b2r": np.ascontiguousarray(b2r),
            }
        )
    return in_maps


def _gather(results):
    y = np.empty((N_TOK, D), dtype=np.float32)
    for core in range(N_CORES):
        out = results[core]["yt"]  # [E_LOC, P, DC, NE] bf16
        for s in range(E_LOC):
            e = E_LOC * core + s
            # yt[s, p, dd, n] = y_e[n, dd*128+p]
            y[e * NE : (e + 1) * NE] = (
                out[s].transpose(2, 1, 0).reshape(NE, D).astype(np.float32)
            )
    return y


def _run(in_maps, **kwargs):
    nc = _get_nc()
    return run_bass_kernel_spmd(nc, in_maps, core_ids=list(range(N_CORES)), **kwargs)


def kernel(xs, fwd_expert_count, w1, b1, w2, b2):
    # fwd_expert_count is uniform (N_TOK // E per expert) by construction,
    # matching the reference, which also hardcodes the uniform grouping.
    in_maps = _prep_in_maps(xs, w1, b1, w2, b2)
    res = _run(in_maps)
    return _gather(res.results)
